# revision 1
# baseline (speedup 1.0000x reference)
"""Trainium2 Bass kernel for DeformAxialDW.

Reference computes: out = x + convH(x) + convW(x) where convH/convW are
depthwise 1D convs (7 taps) along H/W with fractional dilation r realized
as bilinear sampling. Expanding the bilinear interpolation over integer
shifts, each conv becomes a per-channel banded (Toeplitz) conv with
2S+1 integer taps, S = floor(3*r)+1.

Per-core plan (1 batch item per NeuronCore, 8 cores):
  - layout: h on SBUF partitions, w in free dim; x split into two aligned
    112-row blocks (rows 0:112 and 112:224), one pair of tiles per channel
  - H-conv: per-channel banded Toeplitz stationary (bf16) x moving (bf16)
    matmuls into fp32 PSUM; cross-block halo handled by "edge" matmuls
    whose Toeplitz is zero except a small corner
  - W-conv: PE-transpose 112x112 blocks of x, then matmul with the
    transposed block as stationary and the per-channel W-Toeplitz as
    moving, accumulated into the SAME PSUM tile as the H-conv
  - identity (+x): fp32 add on VectorE while copying PSUM->SBUF
  - fp32->bf16 casts on GpSimd, PSUM->SBUF transpose copies on ScalarE
"""

import sys

import numpy as np

sys.path.insert(0, "/opt/trn_rl_repo")

import ml_dtypes

BF16 = ml_dtypes.bfloat16

C, H, W = 128, 224, 224
B = 8
HS = 112  # row-block / h_out / w_in block size

_CACHE = {}


def _tap_coeffs(w_taps: np.ndarray, r_val: float, S: int) -> np.ndarray:
    """Expand 7 fractional-dilation taps into 2S+1 integer-shift coeffs."""
    Cn, K = w_taps.shape
    P = K // 2
    alpha = np.zeros((Cn, 2 * S + 1), dtype=np.float64)
    for i in range(K):
        k_pos = i - P
        delta = np.float32(k_pos) * np.float32(r_val)
        d0 = int(np.floor(delta))
        frac = float(np.float32(delta) - np.float32(d0))
        alpha[:, d0 + S] += (1.0 - frac) * w_taps[:, i].astype(np.float64)
        alpha[:, d0 + 1 + S] += frac * w_taps[:, i].astype(np.float64)
    return alpha


def _banded(alpha: np.ndarray, rows: int, cols: int, diag_off: int, S: int):
    """M[i, c, jj] = alpha[c, (i - jj + diag_off)] where |i-jj+diag_off|<=S."""
    Cn = alpha.shape[0]
    out = np.zeros((rows, Cn, cols), dtype=np.float64)
    i = np.arange(rows)[:, None]
    jj = np.arange(cols)[None, :]
    d = i - jj + diag_off
    mask = np.abs(d) <= S
    ii, jjj = np.nonzero(mask)
    out[ii, :, jjj] = alpha[:, d[ii, jjj] + S].T
    return out


def _build_nc(S: int, repeat: int = 1):
    import concourse.mybir as mybir
    from concourse import bacc
    from concourse.tile import TileContext

    f32 = mybir.dt.float32
    bf16 = mybir.dt.bfloat16

    nc = bacc.Bacc("TRN2", target_bir_lowering=False, debug=False)
    x_p = nc.declare_dram_parameter("x", [C, H, W], f32, isOutput=False)
    gh_p = nc.declare_dram_parameter("gh", [HS, C, HS], bf16, isOutput=False)
    gw_p = nc.declare_dram_parameter("gw", [HS, C, HS + 3 * S], bf16, isOutput=False)
    # corner (cross-block halo) stationaries for the H-conv edge matmuls:
    # ce0: h_in block1 rows [112,144) -> h_out [96,112);
    # ce1: h_in block0 rows [64,112) -> h_out [112,128)
    ce0_p = nc.declare_dram_parameter("ce0", [32, C, 16], bf16, isOutput=False)
    ce1_p = nc.declare_dram_parameter("ce1", [48, C, 16], bf16, isOutput=False)
    id_p = nc.declare_dram_parameter("ident", [HS, HS], bf16, isOutput=False)
    out_p = nc.declare_dram_parameter("out", [C, H, W], f32, isOutput=True)

    G = 8  # channels per DMA group
    with TileContext(nc) as tc:
        with tc.tile_pool(name="const", bufs=1) as constp, \
             tc.tile_pool(name="xf", bufs=3) as xfp, \
             tc.tile_pool(name="xb", bufs=3) as xbp, \
             tc.tile_pool(name="gt", bufs=3) as gtp, \
             tc.tile_pool(name="xt", bufs=6) as xtp, \
             tc.tile_pool(name="outs", bufs=3) as outp, \
             tc.tile_pool(name="pp", bufs=4, space="PSUM") as ppp, \
             tc.tile_pool(name="po", bufs=4, space="PSUM") as pop:
            ident = constp.tile([HS, HS], bf16)
            nc.sync.dma_start(out=ident[:, :], in_=id_p[:, :])
            for _rep in range(repeat):
              for c0 in range(0, C, G):
                  ghg = gtp.tile([HS, G, HS], bf16, tag="gh")
                  gwg = gtp.tile([HS, G, HS + 3 * S], bf16, tag="gw")
                  nc.sync.dma_start(out=ghg[:, :, :], in_=gh_p[:, c0:c0 + G, :])
                  nc.sync.dma_start(out=gwg[:, :, :], in_=gw_p[:, c0:c0 + G, :])
                  ce0g = gtp.tile([32, G, 16], bf16, tag="ce0")
                  ce1g = gtp.tile([HS, G, 16], bf16, tag="ce1")
                  nc.sync.dma_start(out=ce0g[:, :, :], in_=ce0_p[:, c0:c0 + G, :])
                  # ce1 occupies partitions [64,112) so the matmul reading
                  # xb[0][64:112] sees matching base partitions
                  nc.sync.dma_start(out=ce1g[64:HS, :, :], in_=ce1_p[:, c0:c0 + G, :])
                  xf = []
                  xb = []
                  for t in (0, 1):
                      xf_t = xfp.tile([HS, G, W], f32, tag=f"xf{t}")
                      nc.sync.dma_start(
                          out=xf_t[:, :, :],
                          in_=x_p[c0:c0 + G, t * HS:(t + 1) * HS, :].rearrange(
                              "c h w -> h c w"
                          ),
                      )
                      xb_t = xbp.tile([HS, G, W], bf16, tag=f"xb{t}")
                      nc.gpsimd.tensor_copy(out=xb_t[:, :, :], in_=xf_t[:, :, :])
                      xf.append(xf_t)
                      xb.append(xb_t)
                  og0 = outp.tile([HS, G, W], f32, tag="ot0")
                  og1 = outp.tile([HS, G, W], f32, tag="ot1")
                  og = [og0, og1]
                  for cl in range(G):
                      # transpose x blocks: xts[q][:, t, :] = x[tblock_t, wchunk_q].T
                      xts = []
                      for q in (0, 1):
                          xt_t = xtp.tile([HS, 2, HS], bf16, tag=f"xt{q}")
                          pp = ppp.tile([HS, 2, HS], bf16)
                          for t in (0, 1):
                              nc.tensor.matmul(
                                  out=pp[:, t, :],
                                  lhsT=xb[t][0:HS, cl, q * HS:(q + 1) * HS],
                                  rhs=ident[:, :],
                                  is_transpose=True,
                                  skip_group_check=True,
                              )
                          nc.scalar.copy(out=xt_t[:, :, :], in_=pp[:, :, :])
                          xts.append(xt_t)
                      for t in (0, 1):
                          po = pop.tile([HS, W], f32)
                          # H-conv: main (same-block) + edge (other block)
                          nc.tensor.matmul(
                              out=po[:, :],
                              lhsT=ghg[0:HS, cl, :],
                              rhs=xb[t][0:HS, cl, :],
                              start=True, stop=False,
                          )
                          if t == 0:
                              nc.tensor.matmul(
                                  out=po[96:HS, :],
                                  lhsT=ce0g[0:32, cl, :],
                                  rhs=xb[1][0:32, cl, :],
                                  start=False, stop=False,
                                  tile_position=(0, 96),
                              )
                          else:
                              nc.tensor.matmul(
                                  out=po[0:16, :],
                                  lhsT=ce1g[64:HS, cl, :],
                                  rhs=xb[0][64:HS, cl, :],
                                  start=False, stop=False,
                              )
                          # W-conv: two w_in chunks
                          nc.tensor.matmul(
                              out=po[0:HS, 0:HS + S],
                              lhsT=xts[0][0:HS, t, :],
                              rhs=gwg[0:HS, cl, 2 * S:3 * S + HS],
                              start=False, stop=False,
                          )
                          nc.tensor.matmul(
                              out=po[0:HS, HS - S:W],
                              lhsT=xts[1][0:HS, t, :],
                              rhs=gwg[0:HS, cl, S:2 * S + HS],
                              start=False, stop=True,
                          )
                          nc.vector.tensor_add(
                              out=og[t][:, cl, :], in0=xf[t][0:HS, cl, :], in1=po[:, :]
                          )
                  for t in (0, 1):
                      # stores ride the second HWDGE ring (ACT) so they don't
                      # block the sync-engine load queue
                      nc.scalar.dma_start(
                          out=out_p[c0:c0 + G, t * HS:(t + 1) * HS, :].rearrange(
                              "c h w -> h c w"
                          ),
                          in_=og[t][:, :, :],
                      )
    nc.compile()
    return nc


def _prepare_consts(weight_h, weight_w, r):
    r_val = float(max(np.float32(r), np.float32(1.0)))
    S = int(np.floor(3.0 * r_val)) + 1
    assert S <= 16, f"dilation r={r_val} too large for this kernel (S={S})"
    wh = np.asarray(weight_h)[:, 0, :, 0].astype(np.float64)
    ww = np.asarray(weight_w)[:, 0, 0, :].astype(np.float64)
    ah = _tap_coeffs(wh, r_val, S)
    aw = _tap_coeffs(ww, r_val, S)
    gh = _banded(ah, HS, HS, 0, S).astype(BF16)
    gw = _banded(aw, HS, HS + 3 * S, 2 * S, S).astype(BF16)
    # corner stationaries: ce0[i,c,j] = ah[(112+i)-(96+j)], i in [0,32), j in [0,16)
    # ce1[i,c,j] = ah[(64+i)-(112+j)], i in [0,48), j in [0,16)
    ce0 = _banded(ah, 32, 16, 16, S).astype(BF16)
    ce1 = _banded(ah, 48, 16, -48, S).astype(BF16)
    ident = np.eye(HS, dtype=BF16)
    return S, gh, gw, ce0, ce1, ident


def kernel(x, weight_h, weight_w, r):
    from concourse.bass_utils import run_bass_kernel_spmd

    x = np.asarray(x, dtype=np.float32)
    assert x.shape == (B, C, H, W), x.shape
    S, gh, gw, ce0, ce1, ident = _prepare_consts(weight_h, weight_w, r)

    if S not in _CACHE:
        _CACHE[S] = _build_nc(S)
    nc = _CACHE[S]

    in_maps = [
        {"x": x[b], "gh": gh, "gw": gw, "ce0": ce0, "ce1": ce1, "ident": ident}
        for b in range(B)
    ]
    res = run_bass_kernel_spmd(nc, in_maps, core_ids=list(range(B)))
    out = np.stack([res.results[b]["out"] for b in range(B)], axis=0)
    return out



# revision 6
# speedup vs baseline: 1.3585x; 1.3585x over previous
"""Trainium2 Bass kernel for DeformAxialDW (channel-sharded, bf16 I/O).

Reference: out = x + convH(x) + convW(x); convH/convW are depthwise 1D
convs (7 taps, fractional dilation r via bilinear sampling) along H/W.
Expanding bilinear interpolation over integer shifts, each conv is a
per-channel banded conv with 2S+1 integer taps, S = floor(3*r)+1.

Sharding: 16 channels per NeuronCore x all 8 batch items, so each
channel's band matrices are loaded once and reused for 8 batch images.

Per (channel, batch) pair, H is split into two overlap-discard blocks:
  A: h in [0, 112+S)    exact; rows [0, 112) stored
  B: h in [112-S, 224)  rows [112, 224) stored (first S rows discarded)
This keeps every matmul operand at partition base 0 (PE tile_position
constraint) and folds all halos into the contraction dim. The identity
(+x) is folded into the H band's center tap. W is handled identically
via two overlapping w-chunks after a PE transpose.

All x / out DMA moves bf16 with >=3.5KB contiguous descriptors (DRAM
layouts [c, h, b, w]) to hit full modeled DMA bandwidth; fp32<->bf16
conversion happens on the host. Batch items are processed in groups of
2 so the PSUM->SBUF epilogue copies amortize their fixed access
latency; GPSIMD cannot read PSUM on TRN2, so the copies alternate
between the Activation and DVE engines.
"""

import sys

import numpy as np

sys.path.insert(0, "/opt/trn_rl_repo")

import ml_dtypes

BF16 = ml_dtypes.bfloat16

B, C, H, W = 8, 128, 224, 224
NCORES = 8
CPC = C // NCORES  # channels per core
HS = 112
GP = 2  # batch items per epilogue group

_CACHE = {}


def _tap_coeffs(w_taps: np.ndarray, r_val: float, S: int) -> np.ndarray:
    """Expand 7 fractional-dilation taps into 2S+1 integer-shift coeffs."""
    Cn, K = w_taps.shape
    P = K // 2
    alpha = np.zeros((Cn, 2 * S + 1), dtype=np.float64)
    for i in range(K):
        k_pos = i - P
        delta = np.float32(k_pos) * np.float32(r_val)
        d0 = int(np.floor(delta))
        frac = float(np.float32(delta) - np.float32(d0))
        alpha[:, d0 + S] += (1.0 - frac) * w_taps[:, i].astype(np.float64)
        alpha[:, d0 + 1 + S] += frac * w_taps[:, i].astype(np.float64)
    return alpha


def _band(alpha: np.ndarray, rows: int, cols: int, diag: int, S: int) -> np.ndarray:
    """M[i, c, jj] = alpha[c, i - jj + diag] where 0 <= i - jj + diag <= 2S."""
    Cn = alpha.shape[0]
    out = np.zeros((rows, Cn, cols), dtype=np.float64)
    i = np.arange(rows)[:, None]
    jj = np.arange(cols)[None, :]
    d = i - jj + diag
    mask = (d >= 0) & (d <= 2 * S)
    ii, jjj = np.nonzero(mask)
    out[ii, :, jjj] = alpha[:, d[ii, jjj]].T
    return out


def _prepare_consts(weight_h, weight_w, r):
    r_val = float(max(np.float32(r), np.float32(1.0)))
    S = int(np.floor(3.0 * r_val)) + 1
    assert S <= 8, f"dilation r={r_val} too large for this kernel (S={S})"
    NA = HS + S  # block A/B height (118), also w-chunk width
    RA = HS + 2 * S  # tile0 rows / H-A contraction size (124)
    wh = np.asarray(weight_h)[:, 0, :, 0].astype(np.float64)
    ww = np.asarray(weight_w)[:, 0, 0, :].astype(np.float64)
    ah = _tap_coeffs(wh, r_val, S)
    ah[:, S] += 1.0  # fold the identity (+x) into the H-conv center tap
    aw = _tap_coeffs(ww, r_val, S)
    # H band [RA, C, NA]: block A uses [0:RA, :, 0:NA], block B [0:NA, :, 0:NA]
    wbh = _band(ah, RA, NA, S, S)
    # W band [NA, C, HS+2S]: chunk0 moving = cols [S:S+HS], chunk1 [2S:2S+HS]
    wbw = _band(aw, NA, HS + 2 * S, 2 * S, S)
    # combined, padded to 128 cols: [RA, C, 2, 128]
    wb = np.zeros((RA, C, 2, 128), dtype=np.float64)
    wb[:, :, 0, :NA] = wbh
    wb[:NA, :, 1, : HS + 2 * S] = wbw
    ident = np.eye(NA, dtype=BF16)
    return S, wb.astype(BF16), ident


def _build_nc(S: int):
    import concourse.mybir as mybir
    from concourse import bacc
    from concourse.tile import TileContext

    f32 = mybir.dt.float32
    bf16 = mybir.dt.bfloat16

    NA = HS + S
    RA = HS + 2 * S
    Q1 = HS - S  # start row/col of block/chunk B

    nc = bacc.Bacc("TRN2", target_bir_lowering=False, debug=False)
    x_p = nc.declare_dram_parameter("x", [CPC, H, B, W], bf16, isOutput=False)
    wb_p = nc.declare_dram_parameter("wb", [RA, CPC, 2, 128], bf16, isOutput=False)
    id_p = nc.declare_dram_parameter("ident", [NA, NA], bf16, isOutput=False)
    o_p = nc.declare_dram_parameter("out", [CPC, H, B, W], bf16, isOutput=True)

    # groups of GP batch items: (c, b0) with b0 in {0, 2, 4, 6}
    groups = [(c, b0) for c in range(CPC) for b0 in range(0, B, GP)]
    NG = len(groups)
    GPC = B // GP  # groups per channel

    with TileContext(nc) as tc:
        with tc.tile_pool(name="const", bufs=1) as constp, \
             tc.tile_pool(name="xt", bufs=3) as xtp, \
             tc.tile_pool(name="xT", bufs=4) as xTp, \
             tc.tile_pool(name="og", bufs=2) as ogp, \
             tc.tile_pool(name="pt", bufs=4, space="PSUM") as ptp, \
             tc.tile_pool(name="pc", bufs=2, space="PSUM") as pcp:
            wband = constp.tile([RA, CPC, 2, 128], bf16)
            nc.sync.dma_start(out=wband[:, :, :, :], in_=wb_p[:, :, :, :])
            ident = constp.tile([NA, NA], bf16)
            nc.sync.dma_start(out=ident[:, :], in_=id_p[:, :])

            xt_tiles = {}
            og_tiles = {}
            xT_tiles = {}

            def load_channel(c):
                xt0 = xtp.tile([RA, B, W], bf16, tag="xt0", name=f"xt0_{c}")
                nc.sync.dma_start(out=xt0[:, :, :], in_=x_p[c, 0:RA, :, :])
                xt1 = xtp.tile([NA, B, W], bf16, tag="xt1", name=f"xt1_{c}")
                nc.sync.dma_start(out=xt1[:, :, :], in_=x_p[c, Q1:H, :, :])
                xt_tiles[c] = (xt0, xt1)

            def emit_transposes(j):
                c, b0 = groups[j]
                xt0, xt1 = xt_tiles[c]
                pt = ptp.tile([NA, GP, 4, NA], bf16, tag="pt", name=f"pt_{j}")
                for p in range(GP):
                    for k, (src, q0) in enumerate(
                        ((xt0, 0), (xt0, Q1), (xt1, 0), (xt1, Q1))
                    ):
                        nc.tensor.matmul(
                            out=pt[:, p, k, :],
                            lhsT=src[0:NA, b0 + p, q0:q0 + NA],
                            rhs=ident[:, :],
                            is_transpose=True,
                            skip_group_check=True,
                        )
                xT = xTp.tile([NA, GP, 4, NA], bf16, tag="xT", name=f"xT_{j}")
                if j % 2 == 0:
                    nc.vector.tensor_copy(out=xT[:, :, :, :], in_=pt[:, :, :, :])
                else:
                    nc.scalar.copy(out=xT[:, :, :, :], in_=pt[:, :, :, :])
                xT_tiles[j] = xT

            load_channel(0)
            emit_transposes(0)
            emit_transposes(1)

            for j, (c, b0) in enumerate(groups):
                if b0 == 0 and c + 1 < CPC:
                    load_channel(c + 1)
                if j + 2 < NG:
                    emit_transposes(j + 2)

                xt0, xt1 = xt_tiles[c]
                xT = xT_tiles.pop(j)
                # [NA, pair, block, 256]: pair stride = 2KB = one PSUM bank,
                # so every matmul accumulation group stays inside a bank
                pc = pcp.tile([NA, GP, 2, W], f32, tag="pc", name=f"pc_{j}",
                              padded_shape=[128, GP, 2, 256])
                for p in range(GP):
                    b = b0 + p
                    for t, (xsrc, nrows) in enumerate(((xt0, RA), (xt1, NA))):
                        nc.tensor.matmul(
                            out=pc[0:NA, p, t, :],
                            lhsT=wband[0:nrows, c, 0, 0:NA],
                            rhs=xsrc[0:nrows, b, :],
                            start=True, stop=False,
                            skip_group_check=True,
                        )
                        nc.tensor.matmul(
                            out=pc[0:NA, p, t, 0:HS],
                            lhsT=xT[0:NA, p, 2 * t, :],
                            rhs=wband[0:NA, c, 1, S:S + HS],
                            start=False, stop=False,
                            skip_group_check=True,
                        )
                        nc.tensor.matmul(
                            out=pc[0:NA, p, t, HS:W],
                            lhsT=xT[0:NA, p, 2 * t + 1, :],
                            rhs=wband[0:NA, c, 1, 2 * S:2 * S + HS],
                            start=False, stop=True,
                            skip_group_check=True,
                        )

                if b0 == 0:
                    ogA = ogp.tile([HS, B, W], bf16, tag="ogA", name=f"ogA_{c}")
                    ogB = ogp.tile([NA, B, W], bf16, tag="ogB", name=f"ogB_{c}")
                    og_tiles[c] = (ogA, ogB)
                ogA, ogB = og_tiles[c]
                # copy+cast the kept rows of each block out of PSUM
                if j % 2 == 0:
                    nc.scalar.copy(out=ogA[0:HS, b0:b0 + GP, :],
                                   in_=pc[0:HS, :, 0, :])
                    nc.vector.tensor_copy(out=ogB[0:NA, b0:b0 + GP, :],
                                          in_=pc[0:NA, :, 1, :])
                else:
                    nc.vector.tensor_copy(out=ogA[0:HS, b0:b0 + GP, :],
                                          in_=pc[0:HS, :, 0, :])
                    nc.scalar.copy(out=ogB[0:NA, b0:b0 + GP, :],
                                   in_=pc[0:NA, :, 1, :])

                if b0 == B - GP:
                    nc.scalar.dma_start(out=o_p[c, 0:HS, :, :], in_=ogA[0:HS, :, :])
                    nc.scalar.dma_start(out=o_p[c, HS:H, :, :], in_=ogB[S:NA, :, :])
                    del og_tiles[c]
    nc.compile()
    return nc


def kernel(x, weight_h, weight_w, r):
    from concourse.bass_utils import run_bass_kernel_spmd

    x = np.asarray(x, dtype=np.float32)
    assert x.shape == (B, C, H, W), x.shape
    S, wb, ident = _prepare_consts(weight_h, weight_w, r)

    if S not in _CACHE:
        _CACHE[S] = _build_nc(S)
    nc = _CACHE[S]

    # host-side shard prep: x -> per-core [c, h, b, w] bf16
    xr = np.ascontiguousarray(x.transpose(1, 2, 0, 3)).astype(BF16)  # [C, H, B, W]
    in_maps = []
    for k in range(NCORES):
        ck = slice(k * CPC, (k + 1) * CPC)
        in_maps.append({
            "x": np.ascontiguousarray(xr[ck]),
            "wb": np.ascontiguousarray(wb[:, ck]),
            "ident": ident,
        })
    res = run_bass_kernel_spmd(nc, in_maps, core_ids=list(range(NCORES)))
    # gather: or_k [CPC, H, B, W] -> out [B, C, H, W] fp32
    full = np.concatenate([res.results[k]["out"] for k in range(NCORES)], axis=0)
    out = np.ascontiguousarray(full.transpose(2, 0, 1, 3)).astype(np.float32)
    return out


# revision 14
# speedup vs baseline: 1.7515x; 1.2893x over previous
"""Trainium2 Bass kernel for DeformAxialDW (channel-sharded, bf16 I/O).

Reference: out = x + convH(x) + convW(x); convH/convW are depthwise 1D
convs (7 taps, fractional dilation r via bilinear sampling) along H/W.
Expanding bilinear interpolation over integer shifts, each conv is a
per-channel banded conv with 2S+1 integer taps, S = floor(3*r)+1.

Sharding: 16 channels per NeuronCore x all 8 batch items, so each
channel's band matrices are loaded once and reused for 8 batch images.

Per (channel, batch) pair, H is split into two overlap-discard blocks:
  A: h in [0, 112+S)    exact; rows [0, 112) stored
  B: h in [112-S, 224)  rows [112, 224) stored (first S rows discarded)
This keeps every matmul operand at partition base 0 (PE tile_position
constraint) and folds all halos into the contraction dim. The identity
(+x) is folded into the H band's center tap. W is handled identically
via two overlapping w-chunks after a PE transpose.

All x / out DMA moves bf16 with >=3.5KB contiguous descriptors (DRAM
layouts [c, h, b, w]) to hit full modeled DMA bandwidth; fp32<->bf16
conversion happens on the host. Batch items are processed in groups of
2 so the PSUM->SBUF epilogue copies amortize their fixed access
latency; GPSIMD cannot read PSUM on TRN2, so the copies alternate
between the Activation and DVE engines.
"""

import sys

import numpy as np

sys.path.insert(0, "/opt/trn_rl_repo")

import ml_dtypes

BF16 = ml_dtypes.bfloat16

B, C, H, W = 8, 128, 224, 224
NCORES = 8
CPC = C // NCORES  # channels per core
HS = 112
GP = 2  # batch items per epilogue group

_CACHE = {}


def _tap_coeffs(w_taps: np.ndarray, r_val: float, S: int) -> np.ndarray:
    """Expand 7 fractional-dilation taps into 2S+1 integer-shift coeffs."""
    Cn, K = w_taps.shape
    P = K // 2
    alpha = np.zeros((Cn, 2 * S + 1), dtype=np.float64)
    for i in range(K):
        k_pos = i - P
        delta = np.float32(k_pos) * np.float32(r_val)
        d0 = int(np.floor(delta))
        frac = float(np.float32(delta) - np.float32(d0))
        alpha[:, d0 + S] += (1.0 - frac) * w_taps[:, i].astype(np.float64)
        alpha[:, d0 + 1 + S] += frac * w_taps[:, i].astype(np.float64)
    return alpha


def _band(alpha: np.ndarray, rows: int, cols: int, diag: int, S: int) -> np.ndarray:
    """M[i, c, jj] = alpha[c, i - jj + diag] where 0 <= i - jj + diag <= 2S."""
    Cn = alpha.shape[0]
    out = np.zeros((rows, Cn, cols), dtype=np.float64)
    i = np.arange(rows)[:, None]
    jj = np.arange(cols)[None, :]
    d = i - jj + diag
    mask = (d >= 0) & (d <= 2 * S)
    ii, jjj = np.nonzero(mask)
    out[ii, :, jjj] = alpha[:, d[ii, jjj]].T
    return out


def _prepare_consts(weight_h, weight_w, r):
    r_val = float(max(np.float32(r), np.float32(1.0)))
    S = int(np.floor(3.0 * r_val)) + 1
    assert S <= 8, f"dilation r={r_val} too large for this kernel (S={S})"
    NA = HS + S  # block A/B height (118), also w-chunk width
    RA = HS + 2 * S  # tile0 rows / H-A contraction size (124)
    wh = np.asarray(weight_h)[:, 0, :, 0].astype(np.float64)
    ww = np.asarray(weight_w)[:, 0, 0, :].astype(np.float64)
    ah = _tap_coeffs(wh, r_val, S)
    ah[:, S] += 1.0  # fold the identity (+x) into the H-conv center tap
    aw = _tap_coeffs(ww, r_val, S)
    # H band [RA, C, NA]: block A uses [0:RA, :, 0:NA], block B [0:NA, :, 0:NA]
    wbh = _band(ah, RA, NA, S, S)
    # W band [NA, C, HS+2S]: chunk0 moving = cols [S:S+HS], chunk1 [2S:2S+HS]
    wbw = _band(aw, NA, HS + 2 * S, 2 * S, S)
    # combined, padded to 128 cols: [RA, C, 2, 128]
    wb = np.zeros((RA, C, 2, 128), dtype=np.float64)
    wb[:, :, 0, :NA] = wbh
    wb[:NA, :, 1, : HS + 2 * S] = wbw
    ident = np.eye(NA, dtype=BF16)
    return S, wb.astype(BF16), ident


def _build_nc(S: int):
    import concourse.mybir as mybir
    from concourse import bacc
    from concourse.tile import TileContext

    f32 = mybir.dt.float32
    bf16 = mybir.dt.bfloat16

    NA = HS + S
    RA = HS + 2 * S
    Q1 = HS - S  # start row/col of block/chunk B

    nc = bacc.Bacc("TRN2", target_bir_lowering=False, debug=False)
    x_p = nc.declare_dram_parameter("x", [CPC, H, B, W], bf16, isOutput=False)
    wb_p = nc.declare_dram_parameter("wb", [RA, CPC, 2, 128], bf16, isOutput=False)
    id_p = nc.declare_dram_parameter("ident", [NA, NA], bf16, isOutput=False)
    o_p = nc.declare_dram_parameter("out", [CPC, NA, B, 2, W], bf16, isOutput=True)

    # groups of GP batch items: (c, b0) with b0 in {0, 2, 4, 6}
    groups = [(c, b0) for c in range(CPC) for b0 in range(0, B, GP)]
    NG = len(groups)
    GPC = B // GP  # groups per channel

    with TileContext(nc) as tc:
        with tc.tile_pool(name="const", bufs=1) as constp, \
             tc.tile_pool(name="xt", bufs=5) as xtp, \
             tc.tile_pool(name="xT", bufs=4) as xTp, \
             tc.tile_pool(name="og", bufs=2) as ogp, \
             tc.tile_pool(name="pt", bufs=4, space="PSUM") as ptp, \
             tc.tile_pool(name="pc", bufs=2, space="PSUM") as pcp:
            wband = constp.tile([RA, CPC, 2, 128], bf16)
            nc.sync.dma_start(out=wband[:, :, :, :], in_=wb_p[:, :, :, :])
            ident = constp.tile([NA, NA], bf16)
            nc.sync.dma_start(out=ident[:, :], in_=id_p[:, :])

            xt_tiles = {}
            og_tiles = {}
            xT_tiles = {}

            def load_channel(c):
                xt0 = xtp.tile([RA, B, W], bf16, tag="xt0", name=f"xt0_{c}")
                nc.sync.dma_start(out=xt0[:, :, :], in_=x_p[c, 0:RA, :, :])
                xt1 = xtp.tile([NA, B, W], bf16, tag="xt1", name=f"xt1_{c}")
                nc.sync.dma_start(out=xt1[:, :, :], in_=x_p[c, Q1:H, :, :])
                xt_tiles[c] = (xt0, xt1)

            def emit_transposes(j):
                c, b0 = groups[j]
                xt0, xt1 = xt_tiles[c]
                pt = ptp.tile([NA, GP, 4, NA], bf16, tag="pt", name=f"pt_{j}")
                for p in range(GP):
                    for k, (src, q0) in enumerate(
                        ((xt0, 0), (xt0, Q1), (xt1, 0), (xt1, Q1))
                    ):
                        nc.tensor.matmul(
                            out=pt[:, p, k, :],
                            lhsT=src[0:NA, b0 + p, q0:q0 + NA],
                            rhs=ident[:, :],
                            is_transpose=True,
                            skip_group_check=True,
                        )
                xT = xTp.tile([NA, GP, 4, NA], bf16, tag="xT", name=f"xT_{j}")
                nc.vector.tensor_copy(out=xT[:, :, :, :], in_=pt[:, :, :, :])
                xT_tiles[j] = xT

            load_channel(0)
            load_channel(1)
            emit_transposes(0)
            emit_transposes(1)

            for j, (c, b0) in enumerate(groups):
                if b0 == 0 and c + 2 < CPC:
                    load_channel(c + 2)
                if j + 2 < NG:
                    emit_transposes(j + 2)

                xt0, xt1 = xt_tiles[c]
                xT = xT_tiles.pop(j)
                # [NA, pair, block, 256]: pair stride = 2KB = one PSUM bank,
                # so every matmul accumulation group stays inside a bank
                pc = pcp.tile([NA, GP, 2, W], f32, tag="pc", name=f"pc_{j}",
                              padded_shape=[128, GP, 2, 256])
                for p in range(GP):
                    b = b0 + p
                    for t, (xsrc, nrows) in enumerate(((xt0, RA), (xt1, NA))):
                        nc.tensor.matmul(
                            out=pc[0:NA, p, t, :],
                            lhsT=wband[0:nrows, c, 0, 0:NA],
                            rhs=xsrc[0:nrows, b, :],
                            start=True, stop=False,
                            skip_group_check=True,
                        )
                        nc.tensor.matmul(
                            out=pc[0:NA, p, t, 0:HS],
                            lhsT=xT[0:NA, p, 2 * t, :],
                            rhs=wband[0:NA, c, 1, S:S + HS],
                            start=False, stop=False,
                            skip_group_check=True,
                        )
                        nc.tensor.matmul(
                            out=pc[0:NA, p, t, HS:W],
                            lhsT=xT[0:NA, p, 2 * t + 1, :],
                            rhs=wband[0:NA, c, 1, 2 * S:2 * S + HS],
                            start=False, stop=True,
                            skip_group_check=True,
                        )

                if b0 == 0:
                    og = ogp.tile([NA, B, 2, W], bf16, tag="og", name=f"og_{c}")
                    og_tiles[c] = og
                og = og_tiles[c]
                # one copy+cast per group: both blocks, both batch items
                # (discarded halo rows ride along and are dropped on host)
                nc.scalar.copy(out=og[0:NA, b0:b0 + GP, :, :],
                               in_=pc[0:NA, :, :, :])

                if b0 == B - GP:
                    nc.scalar.dma_start(out=o_p[c, :, :, :, :], in_=og[0:NA, :, :, :])
                    del og_tiles[c]
    nc.compile()
    return nc


def kernel(x, weight_h, weight_w, r):
    from concourse.bass_utils import run_bass_kernel_spmd

    x = np.asarray(x, dtype=np.float32)
    assert x.shape == (B, C, H, W), x.shape
    S, wb, ident = _prepare_consts(weight_h, weight_w, r)

    if S not in _CACHE:
        _CACHE[S] = _build_nc(S)
    nc = _CACHE[S]

    # host-side shard prep: x -> per-core [c, h, b, w] bf16
    xr = np.ascontiguousarray(x.transpose(1, 2, 0, 3)).astype(BF16)  # [C, H, B, W]
    in_maps = []
    for k in range(NCORES):
        ck = slice(k * CPC, (k + 1) * CPC)
        in_maps.append({
            "x": np.ascontiguousarray(xr[ck]),
            "wb": np.ascontiguousarray(wb[:, ck]),
            "ident": ident,
        })
    res = run_bass_kernel_spmd(nc, in_maps, core_ids=list(range(NCORES)))
    # gather: or_k [CPC, NA, B, 2, W]; block A rows [0:112) are h [0:112),
    # block B rows [S:NA) are h [112:224)
    NA = HS + S
    full = np.concatenate([res.results[k]["out"] for k in range(NCORES)], axis=0)
    out = np.empty((B, C, H, W), dtype=np.float32)
    out[:, :, :HS, :] = full[:, 0:HS, :, 0, :].transpose(2, 0, 1, 3)
    out[:, :, HS:, :] = full[:, S:NA, :, 1, :].transpose(2, 0, 1, 3)
    return out


# revision 16
# speedup vs baseline: 1.8141x; 1.0357x over previous
"""Trainium2 Bass kernel for DeformAxialDW (channel-sharded, bf16 I/O).

Reference: out = x + convH(x) + convW(x); convH/convW are depthwise 1D
convs (7 taps, fractional dilation r via bilinear sampling) along H/W.
Expanding bilinear interpolation over integer shifts, each conv is a
per-channel banded conv with 2S+1 integer taps, S = floor(3*r)+1.

Sharding: 16 channels per NeuronCore x all 8 batch items, so each
channel's band matrices are loaded once and reused for 8 batch images.

Per (channel, batch) pair, H is split into two overlap-discard blocks:
  A: h in [0, 112+S)    exact; rows [0, 112) stored
  B: h in [112-S, 224)  rows [112, 224) stored (first S rows discarded)
This keeps every matmul operand at partition base 0 (PE tile_position
constraint) and folds all halos into the contraction dim. The identity
(+x) is folded into the H band's center tap. W is handled identically
via two overlapping w-chunks after a PE transpose.

All x / out DMA moves bf16 with >=3.5KB contiguous descriptors (DRAM
layouts [c, h, b, w]) to hit full modeled DMA bandwidth; fp32<->bf16
conversion happens on the host. Batch items are processed in groups of
2 so the PSUM->SBUF epilogue copies amortize their fixed access
latency; GPSIMD cannot read PSUM on TRN2, so the copies alternate
between the Activation and DVE engines.
"""

import sys

import numpy as np

sys.path.insert(0, "/opt/trn_rl_repo")

import ml_dtypes

BF16 = ml_dtypes.bfloat16

B, C, H, W = 8, 128, 224, 224
NCORES = 8
CPC = C // NCORES  # channels per core
HS = 112
GP = 2  # batch items per epilogue group

_CACHE = {}


def _tap_coeffs(w_taps: np.ndarray, r_val: float, S: int) -> np.ndarray:
    """Expand 7 fractional-dilation taps into 2S+1 integer-shift coeffs."""
    Cn, K = w_taps.shape
    P = K // 2
    alpha = np.zeros((Cn, 2 * S + 1), dtype=np.float64)
    for i in range(K):
        k_pos = i - P
        delta = np.float32(k_pos) * np.float32(r_val)
        d0 = int(np.floor(delta))
        frac = float(np.float32(delta) - np.float32(d0))
        alpha[:, d0 + S] += (1.0 - frac) * w_taps[:, i].astype(np.float64)
        alpha[:, d0 + 1 + S] += frac * w_taps[:, i].astype(np.float64)
    return alpha


def _band(alpha: np.ndarray, rows: int, cols: int, diag: int, S: int) -> np.ndarray:
    """M[i, c, jj] = alpha[c, i - jj + diag] where 0 <= i - jj + diag <= 2S."""
    Cn = alpha.shape[0]
    out = np.zeros((rows, Cn, cols), dtype=np.float64)
    i = np.arange(rows)[:, None]
    jj = np.arange(cols)[None, :]
    d = i - jj + diag
    mask = (d >= 0) & (d <= 2 * S)
    ii, jjj = np.nonzero(mask)
    out[ii, :, jjj] = alpha[:, d[ii, jjj]].T
    return out


def _prepare_consts(weight_h, weight_w, r):
    r_val = float(max(np.float32(r), np.float32(1.0)))
    S = int(np.floor(3.0 * r_val)) + 1
    assert S <= 8, f"dilation r={r_val} too large for this kernel (S={S})"
    NA = HS + S  # block A/B height (118), also w-chunk width
    RA = HS + 2 * S  # tile0 rows / H-A contraction size (124)
    wh = np.asarray(weight_h)[:, 0, :, 0].astype(np.float64)
    ww = np.asarray(weight_w)[:, 0, 0, :].astype(np.float64)
    ah = _tap_coeffs(wh, r_val, S)
    ah[:, S] += 1.0  # fold the identity (+x) into the H-conv center tap
    aw = _tap_coeffs(ww, r_val, S)
    # H band [NA, C, NA]: block A uses cols [0:HS), block B cols [0:NA)
    wbh = _band(ah, NA, NA, S, S)
    # W band [NA, C, HS+2S]: chunk0 moving = cols [S:S+HS], chunk1 [2S:2S+HS]
    wbw = _band(aw, NA, HS + 2 * S, 2 * S, S)
    # combined, padded to 128 cols: [NA, C, 2, 128]
    wb = np.zeros((NA, C, 2, 128), dtype=np.float64)
    wb[:, :, 0, :NA] = wbh
    wb[:, :, 1, : HS + 2 * S] = wbw
    ident = np.eye(NA, dtype=BF16)
    return S, wb.astype(BF16), ident


def _build_nc(S: int):
    import concourse.mybir as mybir
    from concourse import bacc
    from concourse.tile import TileContext

    f32 = mybir.dt.float32
    bf16 = mybir.dt.bfloat16

    NA = HS + S
    RA = HS + 2 * S
    Q1 = HS - S  # start row/col of block/chunk B

    nc = bacc.Bacc("TRN2", target_bir_lowering=False, debug=False)
    x_p = nc.declare_dram_parameter("x", [CPC, H, B, W], bf16, isOutput=False)
    wb_p = nc.declare_dram_parameter("wb", [NA, CPC, 2, 128], bf16, isOutput=False)
    id_p = nc.declare_dram_parameter("ident", [NA, NA], bf16, isOutput=False)
    o_p = nc.declare_dram_parameter("out", [CPC, H, B, W], bf16, isOutput=True)

    # groups of GP batch items: (c, b0) with b0 in {0, 2, 4, 6}
    groups = [(c, b0) for c in range(CPC) for b0 in range(0, B, GP)]
    NG = len(groups)
    GPC = B // GP  # groups per channel

    with TileContext(nc) as tc:
        with tc.tile_pool(name="const", bufs=1) as constp, \
             tc.tile_pool(name="xt", bufs=5) as xtp, \
             tc.tile_pool(name="xT", bufs=4) as xTp, \
             tc.tile_pool(name="og", bufs=3) as ogp, \
             tc.tile_pool(name="pt", bufs=4, space="PSUM") as ptp, \
             tc.tile_pool(name="pc", bufs=2, space="PSUM") as pcp:
            wband = constp.tile([NA, CPC, 2, 128], bf16)
            nc.sync.dma_start(out=wband[:, :, :, :], in_=wb_p[:, :, :, :])
            ident = constp.tile([NA, NA], bf16)
            nc.sync.dma_start(out=ident[:, :], in_=id_p[:, :])

            xt_tiles = {}
            og_tiles = {}
            xT_tiles = {}

            def load_channel(c):
                xt0 = xtp.tile([NA, B, W], bf16, tag="xt0", name=f"xt0_{c}")
                nc.sync.dma_start(out=xt0[:, :, :], in_=x_p[c, 0:NA, :, :])
                xt1 = xtp.tile([NA, B, W], bf16, tag="xt1", name=f"xt1_{c}")
                nc.sync.dma_start(out=xt1[:, :, :], in_=x_p[c, Q1:H, :, :])
                xt_tiles[c] = (xt0, xt1)

            def emit_transposes(j):
                c, b0 = groups[j]
                xt0, xt1 = xt_tiles[c]
                pt = ptp.tile([NA, GP, 4, NA], bf16, tag="pt", name=f"pt_{j}")
                for p in range(GP):
                    for k, (xs, q0, hh) in enumerate(
                        ((xt0, 0, HS), (xt0, Q1, HS), (xt1, 0, NA), (xt1, Q1, NA))
                    ):
                        nc.tensor.matmul(
                            out=pt[:, p, k, 0:hh],
                            lhsT=xs[0:hh, b0 + p, q0:q0 + NA],
                            rhs=ident[0:hh, 0:hh],
                            is_transpose=True,
                            skip_group_check=True,
                        )
                xT = xTp.tile([NA, GP, 4, NA], bf16, tag="xT", name=f"xT_{j}")
                nc.vector.tensor_copy(out=xT[:, :, :, :], in_=pt[:, :, :, :])
                xT_tiles[j] = xT

            load_channel(0)
            load_channel(1)
            emit_transposes(0)
            emit_transposes(1)

            for j, (c, b0) in enumerate(groups):
                if b0 == 0 and c + 2 < CPC:
                    load_channel(c + 2)
                if j + 2 < NG:
                    emit_transposes(j + 2)

                xt0, xt1 = xt_tiles[c]
                xT = xT_tiles.pop(j)
                # [NA, block, pair, 256]: block stride = one PSUM bank, so
                # every matmul accumulation group stays inside a bank
                pc = pcp.tile([NA, 2, GP, W], f32, tag="pc", name=f"pc_{j}",
                              padded_shape=[128, 2, GP, 256])
                for p in range(GP):
                    b = b0 + p
                    for t, (xsrc, hh) in enumerate(((xt0, HS), (xt1, NA))):
                        nc.tensor.matmul(
                            out=pc[0:hh, t, p, :],
                            lhsT=wband[0:NA, c, 0, 0:hh],
                            rhs=xsrc[0:NA, b, :],
                            start=True, stop=False,
                            skip_group_check=True,
                        )
                        nc.tensor.matmul(
                            out=pc[0:hh, t, p, 0:HS],
                            lhsT=xT[0:NA, p, 2 * t, 0:hh],
                            rhs=wband[0:NA, c, 1, S:S + HS],
                            start=False, stop=False,
                            skip_group_check=True,
                        )
                        nc.tensor.matmul(
                            out=pc[0:hh, t, p, HS:W],
                            lhsT=xT[0:NA, p, 2 * t + 1, 0:hh],
                            rhs=wband[0:NA, c, 1, 2 * S:2 * S + HS],
                            start=False, stop=True,
                            skip_group_check=True,
                        )

                if b0 == 0:
                    og = ogp.tile([NA, 2, B, W], bf16, tag="og", name=f"og_{c}")
                    og_tiles[c] = og
                og = og_tiles[c]
                # one copy+cast per group: both blocks, both batch items
                # (discarded halo rows ride along and are dropped at the store)
                nc.scalar.copy(out=og[0:NA, :, b0:b0 + GP, :],
                               in_=pc[0:NA, :, :, :])

                if b0 == B - GP:
                    nc.scalar.dma_start(out=o_p[c, 0:HS, :, :], in_=og[0:HS, 0, :, :])
                    nc.scalar.dma_start(out=o_p[c, HS:H, :, :], in_=og[S:NA, 1, :, :])
                    del og_tiles[c]
    nc.compile()
    return nc


def kernel(x, weight_h, weight_w, r):
    from concourse.bass_utils import run_bass_kernel_spmd

    x = np.asarray(x, dtype=np.float32)
    assert x.shape == (B, C, H, W), x.shape
    S, wb, ident = _prepare_consts(weight_h, weight_w, r)

    if S not in _CACHE:
        _CACHE[S] = _build_nc(S)
    nc = _CACHE[S]

    # host-side shard prep: x -> per-core [c, h, b, w] bf16
    xr = np.ascontiguousarray(x.transpose(1, 2, 0, 3)).astype(BF16)  # [C, H, B, W]
    in_maps = []
    for k in range(NCORES):
        ck = slice(k * CPC, (k + 1) * CPC)
        in_maps.append({
            "x": np.ascontiguousarray(xr[ck]),
            "wb": np.ascontiguousarray(wb[:, ck]),
            "ident": ident,
        })
    res = run_bass_kernel_spmd(nc, in_maps, core_ids=list(range(NCORES)))
    # gather: or_k [CPC, H, B, W] -> out [B, C, H, W] fp32
    full = np.concatenate([res.results[k]["out"] for k in range(NCORES)], axis=0)
    out = np.ascontiguousarray(full.transpose(2, 0, 1, 3)).astype(np.float32)
    return out


# revision 18
# speedup vs baseline: 1.8485x; 1.0190x over previous
"""Trainium2 Bass kernel for DeformAxialDW (channel-sharded, bf16 I/O).

Reference: out = x + convH(x) + convW(x); convH/convW are depthwise 1D
convs (7 taps, fractional dilation r via bilinear sampling) along H/W.
Expanding bilinear interpolation over integer shifts, each conv is a
per-channel banded conv with 2S+1 integer taps, S = floor(3*r)+1.

Sharding: 16 channels per NeuronCore x all 8 batch items, so each
channel's band matrices are loaded once and reused for 8 batch images.

Per (channel, batch) pair, H is split into two overlap-discard blocks:
  A: h in [0, 112+S)    exact; rows [0, 112) stored
  B: h in [112-S, 224)  rows [112, 224) stored (first S rows discarded)
This keeps every matmul operand at partition base 0 (PE tile_position
constraint) and folds all halos into the contraction dim. The identity
(+x) is folded into the H band's center tap. W is handled identically
via two overlapping w-chunks after a PE transpose.

All x / out DMA moves bf16 with >=3.5KB contiguous descriptors (DRAM
layouts [c, h, b, w]) to hit full modeled DMA bandwidth; fp32<->bf16
conversion happens on the host. Batch items are processed in groups of
2 so the PSUM->SBUF epilogue copies amortize their fixed access
latency; GPSIMD cannot read PSUM on TRN2, so the copies alternate
between the Activation and DVE engines.
"""

import sys

import numpy as np

sys.path.insert(0, "/opt/trn_rl_repo")

import ml_dtypes

BF16 = ml_dtypes.bfloat16

B, C, H, W = 8, 128, 224, 224
NCORES = 8
CPC = C // NCORES  # channels per core
HS = 112
GP = 2  # batch items per epilogue group

_CACHE = {}


def _tap_coeffs(w_taps: np.ndarray, r_val: float, S: int) -> np.ndarray:
    """Expand 7 fractional-dilation taps into 2S+1 integer-shift coeffs."""
    Cn, K = w_taps.shape
    P = K // 2
    alpha = np.zeros((Cn, 2 * S + 1), dtype=np.float64)
    for i in range(K):
        k_pos = i - P
        delta = np.float32(k_pos) * np.float32(r_val)
        d0 = int(np.floor(delta))
        frac = float(np.float32(delta) - np.float32(d0))
        alpha[:, d0 + S] += (1.0 - frac) * w_taps[:, i].astype(np.float64)
        alpha[:, d0 + 1 + S] += frac * w_taps[:, i].astype(np.float64)
    return alpha


def _band(alpha: np.ndarray, rows: int, cols: int, diag: int, S: int) -> np.ndarray:
    """M[i, c, jj] = alpha[c, i - jj + diag] where 0 <= i - jj + diag <= 2S."""
    Cn = alpha.shape[0]
    out = np.zeros((rows, Cn, cols), dtype=np.float64)
    i = np.arange(rows)[:, None]
    jj = np.arange(cols)[None, :]
    d = i - jj + diag
    mask = (d >= 0) & (d <= 2 * S)
    ii, jjj = np.nonzero(mask)
    out[ii, :, jjj] = alpha[:, d[ii, jjj]].T
    return out


def _prepare_consts(weight_h, weight_w, r):
    r_val = float(max(np.float32(r), np.float32(1.0)))
    S = int(np.floor(3.0 * r_val)) + 1
    assert S <= 8, f"dilation r={r_val} too large for this kernel (S={S})"
    NA = HS + S  # block A/B height (118), also w-chunk width
    RA = HS + 2 * S  # tile0 rows / H-A contraction size (124)
    wh = np.asarray(weight_h)[:, 0, :, 0].astype(np.float64)
    ww = np.asarray(weight_w)[:, 0, 0, :].astype(np.float64)
    ah = _tap_coeffs(wh, r_val, S)
    ah[:, S] += 1.0  # fold the identity (+x) into the H-conv center tap
    aw = _tap_coeffs(ww, r_val, S)
    # H band [NA, C, NA]: block A uses cols [0:HS), block B cols [0:NA)
    wbh = _band(ah, NA, NA, S, S)
    # W band [NA, C, HS+2S]: chunk0 moving = cols [S:S+HS], chunk1 [2S:2S+HS]
    wbw = _band(aw, NA, HS + 2 * S, 2 * S, S)
    # combined, padded to 128 cols: [NA, C, 2, 128]
    wb = np.zeros((NA, C, 2, 128), dtype=np.float64)
    wb[:, :, 0, :NA] = wbh
    wb[:, :, 1, : HS + 2 * S] = wbw
    ident = np.eye(NA, dtype=BF16)
    return S, wb.astype(BF16), ident


def _build_nc(S: int):
    import concourse.mybir as mybir
    from concourse import bacc
    from concourse.tile import TileContext

    f32 = mybir.dt.float32
    bf16 = mybir.dt.bfloat16

    NA = HS + S
    RA = HS + 2 * S
    Q1 = HS - S  # start row/col of block/chunk B

    nc = bacc.Bacc("TRN2", target_bir_lowering=False, debug=False)
    x_p = nc.declare_dram_parameter("x", [CPC, H, B, W], bf16, isOutput=False)
    wb_p = nc.declare_dram_parameter("wb", [NA, CPC, 2, 128], bf16, isOutput=False)
    id_p = nc.declare_dram_parameter("ident", [NA, NA], bf16, isOutput=False)
    o_p = nc.declare_dram_parameter("out", [CPC, H, B, W], bf16, isOutput=True)

    # groups of GP batch items: (c, b0) with b0 in {0, 2, 4, 6}
    groups = [(c, b0) for c in range(CPC) for b0 in range(0, B, GP)]
    NG = len(groups)
    GPC = B // GP  # groups per channel

    with TileContext(nc) as tc:
        with tc.tile_pool(name="const", bufs=1) as constp, \
             tc.tile_pool(name="xt", bufs=5) as xtp, \
             tc.tile_pool(name="xT", bufs=4) as xTp, \
             tc.tile_pool(name="og", bufs=3) as ogp, \
             tc.tile_pool(name="pt", bufs=3, space="PSUM") as ptp, \
             tc.tile_pool(name="pc", bufs=2, space="PSUM") as pcp:
            ident = constp.tile([NA, NA], bf16)
            nc.sync.dma_start(out=ident[:, :], in_=id_p[:, :])
            wband = constp.tile([NA, CPC, 2, 128], bf16)

            xt_tiles = {}
            og_tiles = {}
            xT_tiles = {}

            def load_channel(c):
                xt0 = xtp.tile([NA, B, W], bf16, tag="xt0", name=f"xt0_{c}")
                nc.sync.dma_start(out=xt0[:, :, :], in_=x_p[c, 0:NA, :, :])
                xt1 = xtp.tile([NA, B, W], bf16, tag="xt1", name=f"xt1_{c}")
                nc.sync.dma_start(out=xt1[:, :, :], in_=x_p[c, Q1:H, :, :])
                xt_tiles[c] = (xt0, xt1)

            def emit_transposes(j):
                c, b0 = groups[j]
                xt0, xt1 = xt_tiles[c]
                pt = ptp.tile([NA, GP, 4, NA], bf16, tag="pt", name=f"pt_{j}")
                for p in range(GP):
                    for k, (xs, q0, hh) in enumerate(
                        ((xt0, 0, HS), (xt0, Q1, HS), (xt1, 0, NA), (xt1, Q1, NA))
                    ):
                        nc.tensor.matmul(
                            out=pt[:, p, k, 0:hh],
                            lhsT=xs[0:hh, b0 + p, q0:q0 + NA],
                            rhs=ident[0:hh, 0:hh],
                            is_transpose=True,
                            skip_group_check=True,
                        )
                xT = xTp.tile([NA, GP, 4, NA], bf16, tag="xT", name=f"xT_{j}")
                nc.vector.tensor_copy(out=xT[:, :, :, :], in_=pt[:, :, :, :])
                xT_tiles[j] = xT

            load_channel(0)
            nc.sync.dma_start(out=wband[:, :, :, :], in_=wb_p[:, :, :, :])
            load_channel(1)
            # PE p-state warmup: dummy matmuls on the identity while the
            # first x tiles stream in, so real work starts at full clock
            warm = ptp.tile([NA, NA], f32, tag="warm", name="warm", bufs=1)
            for _ in range(40):
                nc.tensor.matmul(out=warm[:, :], lhsT=ident[:, :],
                                 rhs=ident[:, :], skip_group_check=True)
            emit_transposes(0)
            emit_transposes(1)

            for j, (c, b0) in enumerate(groups):
                if b0 == 0 and c + 2 < CPC:
                    load_channel(c + 2)
                if j + 2 < NG:
                    emit_transposes(j + 2)

                xt0, xt1 = xt_tiles[c]
                xT = xT_tiles.pop(j)
                # [NA, block, pair, 256]: block stride = one PSUM bank, so
                # every matmul accumulation group stays inside a bank
                pc = pcp.tile([NA, 2, GP, W], f32, tag="pc", name=f"pc_{j}",
                              padded_shape=[128, 2, GP, 256])
                for p in range(GP):
                    b = b0 + p
                    for t, (xsrc, hh) in enumerate(((xt0, HS), (xt1, NA))):
                        nc.tensor.matmul(
                            out=pc[0:hh, t, p, :],
                            lhsT=wband[0:NA, c, 0, 0:hh],
                            rhs=xsrc[0:NA, b, :],
                            start=True, stop=False,
                            skip_group_check=True,
                        )
                        nc.tensor.matmul(
                            out=pc[0:hh, t, p, 0:HS],
                            lhsT=xT[0:NA, p, 2 * t, 0:hh],
                            rhs=wband[0:NA, c, 1, S:S + HS],
                            start=False, stop=False,
                            skip_group_check=True,
                        )
                        nc.tensor.matmul(
                            out=pc[0:hh, t, p, HS:W],
                            lhsT=xT[0:NA, p, 2 * t + 1, 0:hh],
                            rhs=wband[0:NA, c, 1, 2 * S:2 * S + HS],
                            start=False, stop=True,
                            skip_group_check=True,
                        )

                if b0 == 0:
                    og = ogp.tile([NA, 2, B, W], bf16, tag="og", name=f"og_{c}")
                    og_tiles[c] = og
                og = og_tiles[c]
                # one copy+cast per group: both blocks, both batch items
                # (discarded halo rows ride along and are dropped at the store)
                nc.scalar.copy(out=og[0:NA, :, b0:b0 + GP, :],
                               in_=pc[0:NA, :, :, :])

                if b0 == B - GP:
                    nc.scalar.dma_start(out=o_p[c, 0:HS, :, :], in_=og[0:HS, 0, :, :])
                    nc.scalar.dma_start(out=o_p[c, HS:H, :, :], in_=og[S:NA, 1, :, :])
                    del og_tiles[c]
    nc.compile()
    return nc


def kernel(x, weight_h, weight_w, r):
    from concourse.bass_utils import run_bass_kernel_spmd

    x = np.asarray(x, dtype=np.float32)
    assert x.shape == (B, C, H, W), x.shape
    S, wb, ident = _prepare_consts(weight_h, weight_w, r)

    if S not in _CACHE:
        _CACHE[S] = _build_nc(S)
    nc = _CACHE[S]

    # host-side shard prep: x -> per-core [c, h, b, w] bf16
    xr = np.ascontiguousarray(x.transpose(1, 2, 0, 3)).astype(BF16)  # [C, H, B, W]
    in_maps = []
    for k in range(NCORES):
        ck = slice(k * CPC, (k + 1) * CPC)
        in_maps.append({
            "x": np.ascontiguousarray(xr[ck]),
            "wb": np.ascontiguousarray(wb[:, ck]),
            "ident": ident,
        })
    res = run_bass_kernel_spmd(nc, in_maps, core_ids=list(range(NCORES)))
    # gather: or_k [CPC, H, B, W] -> out [B, C, H, W] fp32
    full = np.concatenate([res.results[k]["out"] for k in range(NCORES)], axis=0)
    out = np.ascontiguousarray(full.transpose(2, 0, 1, 3)).astype(np.float32)
    return out


# revision 19
# speedup vs baseline: 1.9716x; 1.0666x over previous
"""Trainium2 Bass kernel for DeformAxialDW (channel-sharded, bf16 I/O).

Reference: out = x + convH(x) + convW(x); convH/convW are depthwise 1D
convs (7 taps, fractional dilation r via bilinear sampling) along H/W.
Expanding bilinear interpolation over integer shifts, each conv is a
per-channel banded conv with 2S+1 integer taps, S = floor(3*r)+1.

Sharding: 16 channels per NeuronCore x all 8 batch items, so each
channel's band matrices are loaded once and reused for 8 batch images.

Per (channel, batch) pair, H is split into two overlap-discard blocks:
  A: h in [0, 112+S)    exact; rows [0, 112) stored
  B: h in [112-S, 224)  rows [112, 224) stored (first S rows discarded)
This keeps every matmul operand at partition base 0 (PE tile_position
constraint) and folds all halos into the contraction dim. The identity
(+x) is folded into the H band's center tap. W is handled identically
via two overlapping w-chunks after a PE transpose.

All x / out DMA moves bf16 with >=3.5KB contiguous descriptors (DRAM
layouts [c, h, b, w]) to hit full modeled DMA bandwidth; fp32<->bf16
conversion happens on the host. Batch items are processed in groups of
2 so the PSUM->SBUF epilogue copies amortize their fixed access
latency; GPSIMD cannot read PSUM on TRN2, so the copies alternate
between the Activation and DVE engines.
"""

import sys

import numpy as np

sys.path.insert(0, "/opt/trn_rl_repo")

import ml_dtypes

BF16 = ml_dtypes.bfloat16

B, C, H, W = 8, 128, 224, 224
NCORES = 8
CPC = C // NCORES  # channels per core
HS = 112
GP = 2  # batch items per epilogue group

_CACHE = {}


def _tap_coeffs(w_taps: np.ndarray, r_val: float, S: int) -> np.ndarray:
    """Expand 7 fractional-dilation taps into 2S+1 integer-shift coeffs."""
    Cn, K = w_taps.shape
    P = K // 2
    alpha = np.zeros((Cn, 2 * S + 1), dtype=np.float64)
    for i in range(K):
        k_pos = i - P
        delta = np.float32(k_pos) * np.float32(r_val)
        d0 = int(np.floor(delta))
        frac = float(np.float32(delta) - np.float32(d0))
        alpha[:, d0 + S] += (1.0 - frac) * w_taps[:, i].astype(np.float64)
        alpha[:, d0 + 1 + S] += frac * w_taps[:, i].astype(np.float64)
    return alpha


def _band(alpha: np.ndarray, rows: int, cols: int, diag: int, S: int) -> np.ndarray:
    """M[i, c, jj] = alpha[c, i - jj + diag] where 0 <= i - jj + diag <= 2S."""
    Cn = alpha.shape[0]
    out = np.zeros((rows, Cn, cols), dtype=np.float64)
    i = np.arange(rows)[:, None]
    jj = np.arange(cols)[None, :]
    d = i - jj + diag
    mask = (d >= 0) & (d <= 2 * S)
    ii, jjj = np.nonzero(mask)
    out[ii, :, jjj] = alpha[:, d[ii, jjj]].T
    return out


def _prepare_consts(weight_h, weight_w, r):
    r_val = float(max(np.float32(r), np.float32(1.0)))
    S = int(np.floor(3.0 * r_val)) + 1
    assert S <= 8, f"dilation r={r_val} too large for this kernel (S={S})"
    NA = HS + S  # block A/B height (118), also w-chunk width
    RA = HS + 2 * S  # tile0 rows / H-A contraction size (124)
    wh = np.asarray(weight_h)[:, 0, :, 0].astype(np.float64)
    ww = np.asarray(weight_w)[:, 0, 0, :].astype(np.float64)
    ah = _tap_coeffs(wh, r_val, S)
    ah[:, S] += 1.0  # fold the identity (+x) into the H-conv center tap
    aw = _tap_coeffs(ww, r_val, S)
    # H band [NA, C, NA]: block A uses cols [0:HS), block B cols [0:NA)
    wbh = _band(ah, NA, NA, S, S)
    # W band [NA, C, HS+2S]: chunk0 moving = cols [S:S+HS], chunk1 [2S:2S+HS]
    wbw = _band(aw, NA, HS + 2 * S, 2 * S, S)
    # combined, padded to 128 cols: [NA, C, 2, 128]
    wb = np.zeros((NA, C, 2, 128), dtype=np.float64)
    wb[:, :, 0, :NA] = wbh
    wb[:, :, 1, : HS + 2 * S] = wbw
    ident = np.eye(NA, dtype=BF16)
    return S, wb.astype(BF16), ident


def _build_nc(S: int):
    import concourse.mybir as mybir
    from concourse import bacc
    from concourse.tile import TileContext

    f32 = mybir.dt.float32
    bf16 = mybir.dt.bfloat16

    NA = HS + S
    RA = HS + 2 * S
    Q1 = HS - S  # start row/col of block/chunk B

    nc = bacc.Bacc("TRN2", target_bir_lowering=False, debug=False)
    x_p = nc.declare_dram_parameter("x", [CPC, H, B, W], bf16, isOutput=False)
    wb_p = nc.declare_dram_parameter("wb", [NA, CPC, 2, 128], bf16, isOutput=False)
    id_p = nc.declare_dram_parameter("ident", [NA, NA], bf16, isOutput=False)
    o_p = nc.declare_dram_parameter("out", [CPC, H, B, W], bf16, isOutput=True)

    # groups of GP batch items: (c, b0) with b0 in {0, 2, 4, 6}
    groups = [(c, b0) for c in range(CPC) for b0 in range(0, B, GP)]
    NG = len(groups)
    GPC = B // GP  # groups per channel

    with TileContext(nc) as tc:
        with tc.tile_pool(name="const", bufs=1) as constp, \
             tc.tile_pool(name="xt", bufs=5) as xtp, \
             tc.tile_pool(name="xT", bufs=4) as xTp, \
             tc.tile_pool(name="og", bufs=3) as ogp, \
             tc.tile_pool(name="pt", bufs=2, space="PSUM") as ptp, \
             tc.tile_pool(name="pc", bufs=3, space="PSUM") as pcp:
            ident = constp.tile([NA, NA], bf16)
            nc.sync.dma_start(out=ident[:, :], in_=id_p[:, :])
            wband = constp.tile([NA, CPC, 2, 128], bf16)

            xt_tiles = {}
            og_tiles = {}
            xT_tiles = {}

            def load_channel(c):
                xt0 = xtp.tile([NA, B, W], bf16, tag="xt0", name=f"xt0_{c}")
                nc.sync.dma_start(out=xt0[:, :, :], in_=x_p[c, 0:NA, :, :])
                xt1 = xtp.tile([NA, B, W], bf16, tag="xt1", name=f"xt1_{c}")
                nc.sync.dma_start(out=xt1[:, :, :], in_=x_p[c, Q1:H, :, :])
                xt_tiles[c] = (xt0, xt1)

            def emit_transposes(j):
                c, b0 = groups[j]
                xt0, xt1 = xt_tiles[c]
                pt = ptp.tile([NA, GP, 4, NA], bf16, tag="pt", name=f"pt_{j}")
                for p in range(GP):
                    for k, (xs, q0, hh) in enumerate(
                        ((xt0, 0, HS), (xt0, Q1, HS), (xt1, 0, NA), (xt1, Q1, NA))
                    ):
                        nc.tensor.matmul(
                            out=pt[:, p, k, 0:hh],
                            lhsT=xs[0:hh, b0 + p, q0:q0 + NA],
                            rhs=ident[0:hh, 0:hh],
                            is_transpose=True,
                            skip_group_check=True,
                        )
                xT = xTp.tile([NA, GP, 4, NA], bf16, tag="xT", name=f"xT_{j}")
                nc.vector.tensor_copy(out=xT[:, :, :, :], in_=pt[:, :, :, :])
                xT_tiles[j] = xT

            load_channel(0)
            nc.sync.dma_start(out=wband[:, :, :, :], in_=wb_p[:, :, :, :])
            load_channel(1)
            # PE p-state warmup: dummy matmuls on the identity while the
            # first x tiles stream in, so real work starts at full clock
            warm = pcp.tile([NA, GP, W], f32, tag="pcA", name="warm",
                            padded_shape=[128, GP, 256])
            for _ in range(40):
                nc.tensor.matmul(out=warm[0:NA, 0, 0:NA], lhsT=ident[:, :],
                                 rhs=ident[:, :], skip_group_check=True)
            emit_transposes(0)
            emit_transposes(1)

            for j, (c, b0) in enumerate(groups):
                if b0 == 0 and c + 2 < CPC:
                    load_channel(c + 2)
                if j + 2 < NG:
                    emit_transposes(j + 2)

                xt0, xt1 = xt_tiles[c]
                xT = xT_tiles.pop(j)
                # [NA, block, pair, 256]: block stride = one PSUM bank, so
                # every matmul accumulation group stays inside a bank
                pcA = pcp.tile([NA, GP, W], f32, tag="pcA", name=f"pcA_{j}",
                               padded_shape=[128, GP, 256])
                pcB = pcp.tile([NA, GP, W], f32, tag="pcB", name=f"pcB_{j}",
                               padded_shape=[128, GP, 256])
                for p in range(GP):
                    b = b0 + p
                    for t, (xsrc, hh) in enumerate(((xt0, HS), (xt1, NA))):
                        pcx = pcA if t == 0 else pcB
                        nc.tensor.matmul(
                            out=pcx[0:hh, p, :],
                            lhsT=wband[0:NA, c, 0, 0:hh],
                            rhs=xsrc[0:NA, b, :],
                            start=True, stop=False,
                            skip_group_check=True,
                        )
                        nc.tensor.matmul(
                            out=pcx[0:hh, p, 0:HS],
                            lhsT=xT[0:NA, p, 2 * t, 0:hh],
                            rhs=wband[0:NA, c, 1, S:S + HS],
                            start=False, stop=False,
                            skip_group_check=True,
                        )
                        nc.tensor.matmul(
                            out=pcx[0:hh, p, HS:W],
                            lhsT=xT[0:NA, p, 2 * t + 1, 0:hh],
                            rhs=wband[0:NA, c, 1, 2 * S:2 * S + HS],
                            start=False, stop=True,
                            skip_group_check=True,
                        )

                if b0 == 0:
                    og = ogp.tile([NA, 2, B, W], bf16, tag="og", name=f"og_{c}")
                    og_tiles[c] = og
                og = og_tiles[c]
                nc.scalar.copy(out=og[0:HS, 0, b0:b0 + GP, :],
                               in_=pcA[0:HS, :, :])
                if j % 2 == 0:
                    nc.vector.tensor_copy(out=og[0:NA, 1, b0:b0 + GP, :],
                                          in_=pcB[0:NA, :, :])
                else:
                    nc.scalar.copy(out=og[0:NA, 1, b0:b0 + GP, :],
                                   in_=pcB[0:NA, :, :])

                if b0 == B - GP:
                    nc.scalar.dma_start(out=o_p[c, 0:HS, :, :], in_=og[0:HS, 0, :, :])
                    nc.scalar.dma_start(out=o_p[c, HS:H, :, :], in_=og[S:NA, 1, :, :])
                    del og_tiles[c]
    nc.compile()
    return nc


def kernel(x, weight_h, weight_w, r):
    from concourse.bass_utils import run_bass_kernel_spmd

    x = np.asarray(x, dtype=np.float32)
    assert x.shape == (B, C, H, W), x.shape
    S, wb, ident = _prepare_consts(weight_h, weight_w, r)

    if S not in _CACHE:
        _CACHE[S] = _build_nc(S)
    nc = _CACHE[S]

    # host-side shard prep: x -> per-core [c, h, b, w] bf16
    xr = np.ascontiguousarray(x.transpose(1, 2, 0, 3)).astype(BF16)  # [C, H, B, W]
    in_maps = []
    for k in range(NCORES):
        ck = slice(k * CPC, (k + 1) * CPC)
        in_maps.append({
            "x": np.ascontiguousarray(xr[ck]),
            "wb": np.ascontiguousarray(wb[:, ck]),
            "ident": ident,
        })
    res = run_bass_kernel_spmd(nc, in_maps, core_ids=list(range(NCORES)))
    # gather: or_k [CPC, H, B, W] -> out [B, C, H, W] fp32
    full = np.concatenate([res.results[k]["out"] for k in range(NCORES)], axis=0)
    out = np.ascontiguousarray(full.transpose(2, 0, 1, 3)).astype(np.float32)
    return out


# revision 22
# speedup vs baseline: 2.0291x; 1.0292x over previous
"""Trainium2 Bass kernel for DeformAxialDW (channel-sharded, bf16 I/O).

Reference: out = x + convH(x) + convW(x); convH/convW are depthwise 1D
convs (7 taps, fractional dilation r via bilinear sampling) along H/W.
Expanding bilinear interpolation over integer shifts, each conv is a
per-channel banded conv with 2S+1 integer taps, S = floor(3*r)+1.

Sharding: 16 channels per NeuronCore x all 8 batch items, so each
channel's band matrices are loaded once and reused for 8 batch images.

Per (channel, batch) pair, H is split into two overlap-discard blocks:
  A: h in [0, 112+S)    exact; rows [0, 112) stored
  B: h in [112-S, 224)  rows [112, 224) stored (first S rows discarded)
This keeps every matmul operand at partition base 0 (PE tile_position
constraint) and folds all halos into the contraction dim. The identity
(+x) is folded into the H band's center tap. W is handled identically
via two overlapping w-chunks after a PE transpose.

All x / out DMA moves bf16 with >=3.5KB contiguous descriptors (DRAM
layouts [c, h, b, w]) to hit full modeled DMA bandwidth; fp32<->bf16
conversion happens on the host. Batch items are processed in groups of
2 so the PSUM->SBUF epilogue copies amortize their fixed access
latency; GPSIMD cannot read PSUM on TRN2, so the copies alternate
between the Activation and DVE engines.
"""

import sys

import numpy as np

sys.path.insert(0, "/opt/trn_rl_repo")

import ml_dtypes

BF16 = ml_dtypes.bfloat16

B, C, H, W = 8, 128, 224, 224
NCORES = 8
CPC = C // NCORES  # channels per core
HS = 112
GP = 2  # batch items per epilogue group

_CACHE = {}


def _tap_coeffs(w_taps: np.ndarray, r_val: float, S: int) -> np.ndarray:
    """Expand 7 fractional-dilation taps into 2S+1 integer-shift coeffs."""
    Cn, K = w_taps.shape
    P = K // 2
    alpha = np.zeros((Cn, 2 * S + 1), dtype=np.float64)
    for i in range(K):
        k_pos = i - P
        delta = np.float32(k_pos) * np.float32(r_val)
        d0 = int(np.floor(delta))
        frac = float(np.float32(delta) - np.float32(d0))
        alpha[:, d0 + S] += (1.0 - frac) * w_taps[:, i].astype(np.float64)
        alpha[:, d0 + 1 + S] += frac * w_taps[:, i].astype(np.float64)
    return alpha


def _band(alpha: np.ndarray, rows: int, cols: int, diag: int, S: int) -> np.ndarray:
    """M[i, c, jj] = alpha[c, i - jj + diag] where 0 <= i - jj + diag <= 2S."""
    Cn = alpha.shape[0]
    out = np.zeros((rows, Cn, cols), dtype=np.float64)
    i = np.arange(rows)[:, None]
    jj = np.arange(cols)[None, :]
    d = i - jj + diag
    mask = (d >= 0) & (d <= 2 * S)
    ii, jjj = np.nonzero(mask)
    out[ii, :, jjj] = alpha[:, d[ii, jjj]].T
    return out


def _prepare_consts(weight_h, weight_w, r):
    r_val = float(max(np.float32(r), np.float32(1.0)))
    S = int(np.floor(3.0 * r_val)) + 1
    assert S <= 8, f"dilation r={r_val} too large for this kernel (S={S})"
    NA = HS + S  # block A/B height (118), also w-chunk width
    RA = HS + 2 * S  # tile0 rows / H-A contraction size (124)
    wh = np.asarray(weight_h)[:, 0, :, 0].astype(np.float64)
    ww = np.asarray(weight_w)[:, 0, 0, :].astype(np.float64)
    ah = _tap_coeffs(wh, r_val, S)
    ah[:, S] += 1.0  # fold the identity (+x) into the H-conv center tap
    aw = _tap_coeffs(ww, r_val, S)
    # H band [NA, C, NA]: block A uses cols [0:HS), block B cols [0:NA)
    wbh = _band(ah, NA, NA, S, S)
    # W band [NA, C, HS+2S]: chunk0 moving = cols [S:S+HS], chunk1 [2S:2S+HS]
    wbw = _band(aw, NA, HS + 2 * S, 2 * S, S)
    # combined, padded to 128 cols: [NA, C, 2, 128]
    wb = np.zeros((NA, C, 2, 128), dtype=np.float64)
    wb[:, :, 0, :NA] = wbh
    wb[:, :, 1, : HS + 2 * S] = wbw
    ident = np.eye(NA, dtype=BF16)
    return S, wb.astype(BF16), ident


def _build_nc(S: int):
    import concourse.mybir as mybir
    from concourse import bacc
    from concourse.tile import TileContext

    f32 = mybir.dt.float32
    bf16 = mybir.dt.bfloat16

    NA = HS + S
    RA = HS + 2 * S
    Q1 = HS - S  # start row/col of block/chunk B

    nc = bacc.Bacc("TRN2", target_bir_lowering=False, debug=False)
    x_p = nc.declare_dram_parameter("x", [CPC, H, B, W], bf16, isOutput=False)
    wb_p = nc.declare_dram_parameter("wb", [NA, CPC, 2, 128], bf16, isOutput=False)
    id_p = nc.declare_dram_parameter("ident", [NA, NA], bf16, isOutput=False)
    o_p = nc.declare_dram_parameter("out", [CPC, H, B, W], bf16, isOutput=True)

    # groups of GP batch items: (c, b0) with b0 in {0, 2, 4, 6}
    groups = [(c, b0) for c in range(CPC) for b0 in range(0, B, GP)]
    NG = len(groups)
    GPC = B // GP  # groups per channel

    with TileContext(nc) as tc:
        with tc.tile_pool(name="const", bufs=1) as constp, \
             tc.tile_pool(name="xt", bufs=5) as xtp, \
             tc.tile_pool(name="xT", bufs=4) as xTp, \
             tc.tile_pool(name="og", bufs=5) as ogp, \
             tc.tile_pool(name="pt", bufs=3, space="PSUM") as ptp, \
             tc.tile_pool(name="pc", bufs=3, space="PSUM") as pcp:
            ident = constp.tile([NA, NA], bf16)
            nc.sync.dma_start(out=ident[:, :], in_=id_p[:, :])
            wband = constp.tile([NA, CPC, 2, 128], bf16)

            xt_tiles = {}
            og_tiles = {}
            xT_tiles = {}

            def load_channel(c):
                xt0 = xtp.tile([NA, B, W], bf16, tag="xt0", name=f"xt0_{c}")
                nc.sync.dma_start(out=xt0[:, :, :], in_=x_p[c, 0:NA, :, :])
                xt1 = xtp.tile([NA, B, W], bf16, tag="xt1", name=f"xt1_{c}")
                nc.sync.dma_start(out=xt1[:, :, :], in_=x_p[c, Q1:H, :, :])
                xt_tiles[c] = (xt0, xt1)

            def emit_transposes(j):
                c, b0 = groups[j]
                xt0, xt1 = xt_tiles[c]
                pt = ptp.tile([NA, GP, 4, NA], bf16, tag="pt", name=f"pt_{j}")
                for p in range(GP):
                    for k, (xs, q0, hh) in enumerate(
                        ((xt0, 0, HS), (xt0, Q1, HS), (xt1, 0, NA), (xt1, Q1, NA))
                    ):
                        nc.tensor.matmul(
                            out=pt[:, p, k, 0:hh],
                            lhsT=xs[0:hh, b0 + p, q0:q0 + NA],
                            rhs=ident[0:hh, 0:hh],
                            is_transpose=True,
                            skip_group_check=True,
                        )
                xT = xTp.tile([NA, GP, 4, NA], bf16, tag="xT", name=f"xT_{j}")
                nc.vector.tensor_copy(out=xT[:, :, :, :], in_=pt[:, :, :, :])
                xT_tiles[j] = xT

            load_channel(0)
            nc.sync.dma_start(out=wband[:, 0:2, :, :], in_=wb_p[:, 0:2, :, :])
            load_channel(1)
            nc.sync.dma_start(out=wband[:, 2:CPC, :, :], in_=wb_p[:, 2:CPC, :, :])
            # PE p-state warmup: dummy matmuls on the identity while the
            # first x tiles stream in, so real work starts at full clock
            warm = ptp.tile([NA, GP, 4, NA], bf16, tag="pt", name="warm")
            for _ in range(16):
                nc.tensor.matmul(out=warm[0:NA, 0, 0, :], lhsT=ident[:, :],
                                 rhs=ident[:, :], is_transpose=True,
                                 skip_group_check=True)
            emit_transposes(0)
            emit_transposes(1)

            for j, (c, b0) in enumerate(groups):
                if b0 == 0 and c + 2 < CPC:
                    load_channel(c + 2)
                if j + 2 < NG:
                    emit_transposes(j + 2)

                xt0, xt1 = xt_tiles[c]
                xT = xT_tiles.pop(j)
                # [NA, block, pair, 256]: block stride = one PSUM bank, so
                # every matmul accumulation group stays inside a bank
                pcA = pcp.tile([NA, GP, W], f32, tag="pcA", name=f"pcA_{j}",
                               padded_shape=[128, GP, 256], bufs=2)
                pcB = pcp.tile([NA, GP, W], f32, tag="pcB", name=f"pcB_{j}",
                               padded_shape=[128, GP, 256])
                for p in range(GP):
                    b = b0 + p
                    for t, (xsrc, hh) in enumerate(((xt0, HS), (xt1, NA))):
                        pcx = pcA if t == 0 else pcB
                        nc.tensor.matmul(
                            out=pcx[0:hh, p, :],
                            lhsT=wband[0:NA, c, 0, 0:hh],
                            rhs=xsrc[0:NA, b, :],
                            start=True, stop=False,
                            skip_group_check=True,
                        )
                        nc.tensor.matmul(
                            out=pcx[0:hh, p, 0:HS],
                            lhsT=xT[0:NA, p, 2 * t, 0:hh],
                            rhs=wband[0:NA, c, 1, S:S + HS],
                            start=False, stop=False,
                            skip_group_check=True,
                        )
                        nc.tensor.matmul(
                            out=pcx[0:hh, p, HS:W],
                            lhsT=xT[0:NA, p, 2 * t + 1, 0:hh],
                            rhs=wband[0:NA, c, 1, 2 * S:2 * S + HS],
                            start=False, stop=True,
                            skip_group_check=True,
                        )

                if b0 == 0:
                    og = ogp.tile([NA, 2, B, W], bf16, tag="og", name=f"og_{c}")
                    og_tiles[c] = og
                og = og_tiles[c]
                nc.scalar.copy(out=og[0:HS, 0, b0:b0 + GP, :],
                               in_=pcA[0:HS, :, :])
                if j % 2 == 0:
                    nc.vector.tensor_copy(out=og[0:NA, 1, b0:b0 + GP, :],
                                          in_=pcB[0:NA, :, :])
                else:
                    nc.scalar.copy(out=og[0:NA, 1, b0:b0 + GP, :],
                                   in_=pcB[0:NA, :, :])

                if b0 == B - GP:
                    nc.scalar.dma_start(out=o_p[c, 0:HS, :, :], in_=og[0:HS, 0, :, :])
                    nc.scalar.dma_start(out=o_p[c, HS:H, :, :], in_=og[S:NA, 1, :, :])
                    del og_tiles[c]
    nc.compile()
    return nc


def kernel(x, weight_h, weight_w, r):
    from concourse.bass_utils import run_bass_kernel_spmd

    x = np.asarray(x, dtype=np.float32)
    assert x.shape == (B, C, H, W), x.shape
    S, wb, ident = _prepare_consts(weight_h, weight_w, r)

    if S not in _CACHE:
        _CACHE[S] = _build_nc(S)
    nc = _CACHE[S]

    # host-side shard prep: x -> per-core [c, h, b, w] bf16
    xr = np.ascontiguousarray(x.transpose(1, 2, 0, 3)).astype(BF16)  # [C, H, B, W]
    in_maps = []
    for k in range(NCORES):
        ck = slice(k * CPC, (k + 1) * CPC)
        in_maps.append({
            "x": np.ascontiguousarray(xr[ck]),
            "wb": np.ascontiguousarray(wb[:, ck]),
            "ident": ident,
        })
    res = run_bass_kernel_spmd(nc, in_maps, core_ids=list(range(NCORES)))
    # gather: or_k [CPC, H, B, W] -> out [B, C, H, W] fp32
    full = np.concatenate([res.results[k]["out"] for k in range(NCORES)], axis=0)
    out = np.ascontiguousarray(full.transpose(2, 0, 1, 3)).astype(np.float32)
    return out
